# revision 52
# baseline (speedup 1.0000x reference)
"""Trainium2 Bass kernel for nn_Attention_85710367359290 (sparse branch-routed attention).

Semantics (validated vs reference offline, rel err ~0.011):
  q = rope(a @ Wq) per branch (NB=4), k = rope(x @ Wk), v = a @ Wv per branch
  att[b,n,t,s] = q.k/sqrt(C);  amax = max_n exp(att);  p = amax (no max-sub, |att|<~8)
  routing: cmb_n = p * (exp(att_n) >= amax) on causal positions
  y = sum_n cmb_n @ v_n;  Z = sum_s p;  out = (y/Z) @ Wo

Two-phase SPMD over 8 cores (no collectives; host reshuffles between phases).
All tensor data fp16 (PSUM accum f32).

Phase A: k-proj+rope and v-proj, core (b,s4) owns a 512-row s-slice of batch b.
Phase B: q-proj+rope + attention. Core (b,j) owns 4 t-blocks of 128 rows:
  tb = {15-j, 11-j, 7-j, 3-j} assigned to slots 0..3 with fixed per-slot
  s-chunk capacities (16,12,8,4) >= needs (16-j,12-j,8-j,4-j): a uniform
  40-unit program (si-major so each v s-chunk is loaded once).
  Padded units are neutralized by all-zero masks (data).

Unit (si, slot): att[s128, (n4,t128)] = kr_si^T q_slot (4 matmuls ap512 fp16);
  e = exp(att-4) fp16 (Act); amax = max_n e (2 TT DVE); p_m = amax*msk (DVE);
  mb = e>=amax bcast (Pool); cmb = mb*p_m bcast (DVE); Z[:,slot] += colsum
  (PE ones-matmul); y[slot] += v_n^T cmb_n (16 matmuls ap128).
  Epilogue: o = (y/Z)^T @ Wo.

PSUM matmul start=True zeroes the whole bank: only the chronologically first
matmul per bank sets it. DMAs are batched into large strided transfers
(SBUF-resident kr/v/masks) to keep the SP sequencer off the critical path.
"""

import numpy as np
import ml_dtypes

import concourse.bass as bass
import concourse.mybir as mybir
import concourse.tile as tile
from concourse import bacc
from concourse.bass_utils import run_bass_kernel_spmd

F32 = mybir.dt.float32
F16 = mybir.dt.float16
ALU = mybir.AluOpType
ACTF = mybir.ActivationFunctionType
NPF16 = np.float16

B, T, C, NB = 2, 2048, 512, 4
SC = 128          # s-chunk (PSUM partition dim)
BLK = 128         # t-block
NSI = T // SC     # 16
CAPS = (16, 12, 8, 4)
# interleave slots proportionally so consecutive units hit different slots
# (v is SBUF-resident, so any order is legal for the accumulations)
UNITS = sorted(
    [(si, sl) for si in range(NSI) for sl in range(4) if si < CAPS[sl]],
    key=lambda u: ((u[0] + 0.5) / CAPS[u[1]], u[1]))
NU = len(UNITS)   # 40
N_CORES = 8
EXP_BIAS = -4.0

_cache = {}


def build_phase_a():
    if "a" in _cache:
        return _cache["a"]
    nc = bacc.Bacc("TRN2", target_bir_lowering=False, debug=False)

    def din(name, shape, dt):
        return nc.dram_tensor(name, shape, dt, kind="ExternalInput").ap()

    xT = din("xT", [C, 512], F16)       # x[b].T cols of this core's s-slice
    aTv = din("aTv", [C, 512], F16)     # a[b].T same cols (for v)
    Wk = din("Wk", [C, C], F16)         # split-permuted, pre-scaled 1/sqrt(C)
    Wv = din("Wv", [C, NB * C], F16)
    cosA = din("cosA", [C // 2, 512], F16)
    sinA = din("sinA", [C // 2, 512], F16)
    krA = nc.dram_tensor("krA", [C, 512], F16, kind="ExternalOutput").ap()
    vA = nc.dram_tensor("vA", [512, NB * C], F16, kind="ExternalOutput").ap()

    def v4(ap):  # DRAM [512, W] -> [128, (4, W)] chunk-major
        return ap.rearrange("(c p) w -> p c w", p=128)

    with tile.TileContext(nc) as tc:
        with (
            tc.tile_pool(name="pa", bufs=1) as pa,
            tc.tile_pool(name="pat", bufs=4) as pat,
            tc.tile_pool(name="pap", bufs=6, space="PSUM") as pps,
        ):
            xB = pa.tile([128, 4 * 512], F16, tag="xB", name="xB")
            aB = pa.tile([128, 4 * 512], F16, tag="aB", name="aB")
            WkB = pa.tile([128, 4 * C], F16, tag="WkB", name="WkB")
            WvB = pa.tile([128, 4 * NB * C], F16, tag="WvB", name="WvB")
            csB = pa.tile([128, 2 * 512], F16, tag="csB", name="csB")
            snB = pa.tile([128, 2 * 512], F16, tag="snB", name="snB")
            WvBv = WvB.rearrange("p (c n w) -> p c n w", c=4, n=NB)
            Wvv = v4(Wv).rearrange("p c (n w) -> p c n w", n=NB)
            for c in range(4):
                nc.sync.dma_start(
                    out=xB.rearrange("p (c w) -> p c w", c=4)[:, c:c + 1, :],
                    in_=v4(xT)[:, c:c + 1, :])
                nc.sync.dma_start(
                    out=WkB.rearrange("p (c w) -> p c w", c=4)[:, c:c + 1, :],
                    in_=v4(Wk)[:, c:c + 1, :])
            nc.sync.dma_start(out=csB.rearrange("p (c w) -> p c w", c=2),
                              in_=cosA.rearrange("(c p) w -> p c w", p=128))
            nc.sync.dma_start(out=snB.rearrange("p (c w) -> p c w", c=2),
                              in_=sinA.rearrange("(c p) w -> p c w", p=128))
            nc.sync.dma_start(out=aB.rearrange("p (c w) -> p c w", c=4), in_=v4(aTv))
            for n in range(NB):
                nc.sync.dma_start(out=WvBv[:, :, n, :], in_=Wvv[:, :, n, :])
            xTt = [xB[:, i * 512:(i + 1) * 512] for i in range(4)]
            aTt = [aB[:, i * 512:(i + 1) * 512] for i in range(4)]
            WkT = [WkB[:, i * C:(i + 1) * C] for i in range(4)]
            WvT = [WvB[:, i * NB * C:(i + 1) * NB * C] for i in range(4)]
            cst = [csB[:, i * 512:(i + 1) * 512] for i in range(2)]
            snt = [snB[:, i * 512:(i + 1) * 512] for i in range(2)]

            # ---- k proj -> fp16 copy -> rope -> krA ----
            kp16 = [pa.tile([128, 512], F16, tag=f"kp{i}", name=f"kp{i}") for i in range(4)]
            for m in range(4):
                ps = pps.tile([128, 512], F32, tag="pps", name="pps")
                for Kc in range(4):
                    nc.tensor.matmul(ps, WkT[Kc][:, m * 128:(m + 1) * 128], xTt[Kc],
                                     start=(Kc == 0), stop=(Kc == 3))
                nc.scalar.copy(out=kp16[m], in_=ps)
            for h in range(2):
                t1 = pat.tile([128, 512], F16, tag="t1", name="t1")
                t2 = pat.tile([128, 512], F16, tag="t2", name="t2")
                kr = pat.tile([128, 512], F16, tag="kr", name="kr")
                nc.vector.tensor_mul(t1, kp16[h], cst[h])
                nc.vector.tensor_mul(t2, kp16[2 + h], snt[h])
                nc.vector.tensor_sub(kr, t1, t2)
                nc.sync.dma_start(out=krA[h * 128:(h + 1) * 128, :], in_=kr)
                t3 = pat.tile([128, 512], F16, tag="t3", name="t3")
                t4 = pat.tile([128, 512], F16, tag="t4", name="t4")
                kr2 = pat.tile([128, 512], F16, tag="kr2", name="kr2")
                nc.vector.tensor_mul(t3, kp16[h], snt[h])
                nc.vector.tensor_mul(t4, kp16[2 + h], cst[h])
                nc.vector.tensor_add(kr2, t3, t4)
                nc.sync.dma_start(out=krA[(2 + h) * 128:(3 + h) * 128, :], in_=kr2)

            # ---- v proj: vA[s-slice, (n,c)]; nb-outer so Wv(nb0) starts early ----
            vsb = [pa.tile([128, NB * 512], F16, tag=f"vsb{i}", name=f"vsb{i}")
                   for i in range(4)]
            for nb in range(4):
                for sc in range(4):
                    ps = pps.tile([128, 512], F32, tag="pps", name="pps")
                    for Kc in range(4):
                        nc.tensor.matmul(ps, aTt[Kc][:, sc * 128:(sc + 1) * 128],
                                         WvT[Kc][:, nb * 512:(nb + 1) * 512],
                                         start=(Kc == 0), stop=(Kc == 3))
                    dst = vsb[sc][:, nb * 512:(nb + 1) * 512]
                    if (nb + sc) % 2 == 0:
                        nc.scalar.copy(out=dst, in_=ps)
                    else:
                        nc.vector.tensor_scalar_mul(dst, ps, 1.0)
                if nb % 2 == 1:  # ship completed halves early
                    for sc in range(4):
                        nc.sync.dma_start(
                            out=vA[sc * 128:(sc + 1) * 128,
                                   (nb - 1) * 512:(nb + 1) * 512],
                            in_=vsb[sc][:, (nb - 1) * 512:(nb + 1) * 512])
    nc.compile()
    _cache["a"] = nc
    return nc


def build_phase_b():
    if "b" in _cache:
        return _cache["b"]
    nc = bacc.Bacc("TRN2", target_bir_lowering=False, debug=False)

    def din(name, shape, dt):
        return nc.dram_tensor(name, shape, dt, kind="ExternalInput").ap()

    aQ = din("aQ", [C, 512], F16)       # a[b].T cols = 4 t-blocks (slot order)
    Wq = din("Wq", [C, NB * C], F16)    # split-permuted
    cosB = din("cosB", [C // 2, 512], F16)
    sinB = din("sinB", [C // 2, 512], F16)
    krB = din("krB", [C, T], F16)
    vB = din("vB", [T, NB * C], F16)
    WoD = din("Wo", [C, C], F16)
    mskd = din("mskd", [NU, SC, BLK], mybir.dt.float8e4)
    out = nc.dram_tensor("o", [512, C], F32, kind="ExternalOutput").ap()

    with tile.TileContext(nc) as tc:
        with (
            tc.tile_pool(name="persist", bufs=1) as pp,
            tc.tile_pool(name="qtmp", bufs=4) as qtp,
            tc.tile_pool(name="ew", bufs=5) as ew,
            tc.tile_pool(name="ep", bufs=2) as epi,
        ):
            aQb = pp.tile([128, 4 * 512], F16, tag="aQb", name="aQb")
            WqB = pp.tile([128, 4 * NB * C], F16, tag="WqB", name="WqB")
            csB = pp.tile([128, 2 * 512], F16, tag="csB", name="csB")
            snB = pp.tile([128, 2 * 512], F16, tag="snB", name="snB")
            krBt = pp.tile([128, 4 * T], F16, tag="krBt", name="krBt")
            vBt = pp.tile([128, NSI * NB * C], F16, tag="vBt", name="vBt")
            WoB = pp.tile([128, 4 * C], F16, tag="WoB", name="WoB")
            mskB = pp.tile([128, NU * BLK], mybir.dt.float8e4, tag="mskB", name="mskB")
            # qrT[m]: partition = c' in chunk m; free = (n 4, t 512)
            qrT = [pp.tile([128, NB * 512], F16, tag=f"qr{i}", name=f"qr{i}") for i in range(4)]
            ones = pp.tile([128, 1], F16, tag="ones", name="ones")
            ebias = pp.tile([128, 1], F32, tag="ebias", name="ebias")
            nc.vector.memset(ones, 1.0)
            nc.vector.memset(ebias, EXP_BIAS)

            def v4(ap, p=128):  # DRAM [4*p, W] -> [p, (4, W)]
                return ap.rearrange("(c p) w -> p c w", p=p)

            # deliver in consumption order: q-path, kr si-quarters and v per-si
            # chunks interleaved just-in-time; masks in halves; Wo last
            aQv = aQb.rearrange("p (c w) -> p c w", c=4)
            aQs = v4(aQ)
            WqBv = WqB.rearrange("p (c n w) -> p c n w", c=4, n=NB)
            Wqv = v4(Wq).rearrange("p c (n w) -> p c n w", n=NB)
            krv = krBt.rearrange("p (c g w) -> p c g w", c=4, g=4)
            krs = v4(krB).rearrange("p c (g w) -> p c g w", g=4)
            vBv = vB.rearrange("(g p) w -> p g w", p=128)  # [128, 16, 2048]
            vtv = vBt.rearrange("p (g w) -> p g w", g=NSI)
            mskv = mskB.rearrange("p (u w) -> p u w", u=NU)
            msks = mskd.rearrange("u p w -> p u w")

            def vdma(si):
                nc.sync.dma_start(out=vtv[:, si:si + 1, :], in_=vBv[:, si:si + 1, :])

            nc.sync.dma_start(out=aQv[:, 0:2, :], in_=aQs[:, 0:2, :])
            nc.sync.dma_start(out=WqBv[:, 0:2, 0, :], in_=Wqv[:, 0:2, 0, :])
            nc.sync.dma_start(out=aQv[:, 2:4, :], in_=aQs[:, 2:4, :])
            nc.sync.dma_start(out=WqBv[:, 2:4, 0, :], in_=Wqv[:, 2:4, 0, :])
            nc.sync.dma_start(out=krv[:, :, 0, :], in_=krs[:, :, 0, :])
            nc.sync.dma_start(out=WqBv[:, :, 1, :], in_=Wqv[:, :, 1, :])
            vdma(0)
            nc.sync.dma_start(out=csB.rearrange("p (c w) -> p c w", c=2), in_=v4(cosB))
            nc.sync.dma_start(out=snB.rearrange("p (c w) -> p c w", c=2), in_=v4(sinB))
            vdma(1)
            nc.sync.dma_start(out=WqBv[:, :, 2, :], in_=Wqv[:, :, 2, :])
            vdma(2)
            nc.sync.dma_start(out=mskv[:, 0:20, :], in_=msks[:, 0:20, :])
            vdma(3)
            nc.sync.dma_start(out=WqBv[:, :, 3, :], in_=Wqv[:, :, 3, :])
            nc.sync.dma_start(out=krv[:, :, 1, :], in_=krs[:, :, 1, :])
            vdma(4)
            vdma(5)
            nc.sync.dma_start(out=krv[:, :, 2, :], in_=krs[:, :, 2, :])
            vdma(6)
            vdma(7)
            nc.sync.dma_start(out=mskv[:, 20:NU, :], in_=msks[:, 20:NU, :])
            vdma(8)
            vdma(9)
            nc.sync.dma_start(out=krv[:, :, 3, :], in_=krs[:, :, 3, :])
            for si in range(10, NSI):
                vdma(si)
            nc.sync.dma_start(out=WoB.rearrange("p (c w) -> p c w", c=4), in_=v4(WoD))
            aQt = [aQb[:, i * 512:(i + 1) * 512] for i in range(4)]
            WqT = [WqB[:, i * NB * C:(i + 1) * NB * C] for i in range(4)]
            cst = [csB[:, i * 512:(i + 1) * 512] for i in range(2)]
            snt = [snB[:, i * 512:(i + 1) * 512] for i in range(2)]
            krT = [krBt[:, i * T:(i + 1) * T] for i in range(4)]
            WoT = [WoB[:, i * C:(i + 1) * C] for i in range(4)]

            with tc.tile_pool(name="qpp", bufs=8, space="PSUM") as qpp:
                # ---- q proj + rope (all 4 blocks at once) ----
                for n in range(NB):
                    qp16 = [qtp.tile([128, 512], F16, tag=f"qp{m}", name=f"qp{m}")
                            for m in range(4)]
                    for m in range(4):
                        ps = qpp.tile([128, 512], F32, tag="qps", name="qps")
                        for Kc in range(4):
                            nc.tensor.matmul(
                                ps, WqT[Kc][:, (4 * n + m) * 128:(4 * n + m + 1) * 128],
                                aQt[Kc], start=(Kc == 0), stop=(Kc == 3))
                        nc.scalar.copy(out=qp16[m], in_=ps)
                    for h in range(2):
                        eng = nc.vector if h == 0 else nc.gpsimd
                        t1 = qtp.tile([128, 512], F16, tag="qt1", name="qt1")
                        t2 = qtp.tile([128, 512], F16, tag="qt2", name="qt2")
                        eng.tensor_mul(t1, qp16[h], cst[h])
                        eng.tensor_mul(t2, qp16[2 + h], snt[h])
                        nc.vector.tensor_sub(qrT[h][:, n * 512:(n + 1) * 512], t1, t2)
                        t3 = qtp.tile([128, 512], F16, tag="qt3", name="qt3")
                        t4 = qtp.tile([128, 512], F16, tag="qt4", name="qt4")
                        eng.tensor_mul(t3, qp16[h], snt[h])
                        eng.tensor_mul(t4, qp16[2 + h], cst[h])
                        nc.vector.tensor_add(qrT[2 + h][:, n * 512:(n + 1) * 512], t3, t4)

            with tc.tile_pool(name="accp", bufs=1, space="PSUM") as acc:
                yT = [acc.tile([128, 512], F32, tag=f"yT{i}", name=f"yT{i}")
                      for i in range(4)]
                Zp = acc.tile([128, 4], F32, tag="Zp", name="Zp")
                qv = [qrT[Kc].rearrange("p (n t) -> p n t", n=NB) for Kc in range(4)]
                DEPTH = 2  # PE software-pipeline depth: att(u+2) before pv(u)

                with tc.tile_pool(name="attp", bufs=3, space="PSUM") as app:
                    first_u = {sl: min(i for i, u in enumerate(UNITS) if u[1] == sl)
                               for sl in range(4)}
                    last_u = {sl: max(i for i, u in enumerate(UNITS) if u[1] == sl)
                              for sl in range(4)}

                    def emit_epilogue(sl):
                        # o = (y/Z)^T @ Wo; reuse the slot's dead yT bank
                        yb = epi.tile([128, 512], F16, tag="yb", name="yb")
                        nc.scalar.copy(out=yb, in_=yT[sl])
                        zr = epi.tile([128, 1], F32, tag="zr", name="zr")
                        nc.vector.reciprocal(zr, Zp[:, sl:sl + 1])
                        ops = acc.tile([128, 512], F32, tag=f"yT{sl}", name=f"ops{sl}")
                        for Mc in range(4):
                            nc.tensor.matmul(ops, yb[:, Mc * 128:(Mc + 1) * 128],
                                             WoT[Mc], start=(Mc == 0), stop=(Mc == 3))
                        osb = epi.tile([128, 512], F32, tag="osb", name="osb")
                        nc.vector.tensor_scalar_mul(osb, ops, zr[:, 0:1])
                        nc.sync.dma_start(out=out[sl * 128:(sl + 1) * 128, :], in_=osb)

                    def emit_zpv(ui, si, sl, p_m, cmb):
                        start = ui == first_u[sl]
                        stop = ui == last_u[sl]
                        vt = vBt[:, si * NB * C:(si + 1) * NB * C]
                        # start=True zeroes the whole PSUM bank -> only first
                        nc.tensor.matmul(Zp[:, sl:sl + 1], p_m, ones,
                                         start=(ui == 0), stop=stop)
                        for br in range(NB):
                            for Mc in range(4):
                                nc.tensor.matmul(
                                    yT[sl][:, Mc * 128:(Mc + 1) * 128],
                                    vt[:, br * 512 + Mc * 128:br * 512 + (Mc + 1) * 128],
                                    cmb[:, br * 128:(br + 1) * 128],
                                    start=(start and br == 0 and Mc == 0),
                                    stop=(stop and br == 3))
                        if stop:
                            emit_epilogue(sl)

                    pending = []
                    for ui, (si, sl) in enumerate(UNITS):
                        msk = mskB[:, ui * BLK:(ui + 1) * BLK]
                        att = app.tile([128, NB * BLK], F32, tag="att", name="att")
                        attv = att.rearrange("p (n t) -> p n t", n=NB)
                        for bp in range(2):
                            for Kc in range(4):
                                nc.tensor.matmul(
                                    attv[:, bp * 2:(bp + 1) * 2],
                                    krT[Kc][:, si * 128:(si + 1) * 128],
                                    qv[Kc][:, bp * 2:(bp + 1) * 2,
                                           sl * 128:(sl + 1) * 128],
                                    start=(bp == 0 and Kc == 0), stop=(Kc == 3))
                        e = ew.tile([128, NB * BLK], F16, tag="e", name="e")
                        nc.scalar.activation(out=e, in_=att, func=ACTF.Exp,
                                             bias=ebias[:, 0:1])
                        m1 = ew.tile([128, 256], F16, tag="m1", name="m1")
                        amax = ew.tile([128, BLK], F16, tag="amax", name="amax")
                        p_m = ew.tile([128, BLK], F16, tag="p_m", name="p_m")
                        nc.vector.tensor_max(m1, e[:, 0:256], e[:, 256:512])
                        nc.vector.tensor_max(amax, m1[:, 0:128], m1[:, 128:256])
                        nc.vector.tensor_mul(p_m, amax, msk)
                        amax_b = amax.rearrange("p (a t) -> p a t", a=1).to_broadcast(
                            [128, NB, BLK])
                        pm_b = p_m.rearrange("p (a t) -> p a t", a=1).to_broadcast(
                            [128, NB, BLK])
                        mb = ew.tile([128, NB * BLK], F16, tag="mb", name="mb")
                        cmb = ew.tile([128, NB * BLK], F16, tag="cmb", name="cmb")
                        ev = e.rearrange("p (n t) -> p n t", n=NB)
                        nc.vector.tensor_tensor(
                            out=mb.rearrange("p (n t) -> p n t", n=NB),
                            in0=ev, in1=amax_b, op=ALU.is_ge)
                        nc.vector.tensor_mul(
                            cmb.rearrange("p (n t) -> p n t", n=NB)[:, 0:2],
                            mb.rearrange("p (n t) -> p n t", n=NB)[:, 0:2],
                            pm_b[:, 0:2])
                        nc.gpsimd.tensor_mul(
                            cmb.rearrange("p (n t) -> p n t", n=NB)[:, 2:4],
                            mb.rearrange("p (n t) -> p n t", n=NB)[:, 2:4],
                            pm_b[:, 2:4])
                        pending.append((ui, si, sl, p_m, cmb))
                        if len(pending) > DEPTH:
                            emit_zpv(*pending.pop(0))
                    while pending:
                        emit_zpv(*pending.pop(0))
    nc.compile()
    _cache["b"] = nc
    return nc


def _host_prep(Wq, Wk, Wv, Wo, cos, sin):
    split_idx = np.r_[0:C:2, 1:C:2]
    Wq_p = np.ascontiguousarray(
        Wq.reshape(C, NB, C)[:, :, split_idx].reshape(C, NB * C)).astype(NPF16)
    Wk_p = (Wk[:, split_idx] * np.float32(1.0 / np.sqrt(C))).astype(NPF16)
    Wv_h = Wv.astype(NPF16)
    Wo_h = Wo.astype(NPF16)
    cosT = np.ascontiguousarray(cos[:T].T).astype(NPF16)  # [C/2, T]
    sinT = np.ascontiguousarray(sin[:T].T).astype(NPF16)
    return Wq_p, Wk_p, Wv_h, Wo_h, cosT, sinT


def _core_blocks(j):
    return [15 - j, 11 - j, 7 - j, 3 - j]


def _masks(j):
    m = np.zeros((NU, SC, BLK), np.float32)
    tbs = _core_blocks(j)
    tt = np.arange(BLK)[None, :]
    ss = np.arange(SC)[:, None]
    for ui, (si, sl) in enumerate(UNITS):
        t0 = BLK * tbs[sl]
        m[ui] = (t0 + tt) >= (SC * si + ss)
    return m.astype(ml_dtypes.float8_e4m3)


def kernel(a, x, Wq, Wk, Wv, Wo, cos, sin):
    a = np.asarray(a, np.float32)
    x = np.asarray(x, np.float32)
    Wq_p, Wk_p, Wv_h, Wo_h, cosT, sinT = _host_prep(
        np.asarray(Wq, np.float32), np.asarray(Wk, np.float32),
        np.asarray(Wv, np.float32), np.asarray(Wo, np.float32),
        np.asarray(cos, np.float32), np.asarray(sin, np.float32))

    # ---- phase A: k + v ----
    nca = build_phase_a()
    in_a = []
    for core in range(N_CORES):
        b, s4 = divmod(core, 4)
        rows = slice(512 * s4, 512 * (s4 + 1))
        in_a.append({
            "xT": np.ascontiguousarray(x[b].T[:, rows]).astype(NPF16),
            "aTv": np.ascontiguousarray(a[b].T[:, rows]).astype(NPF16),
            "Wk": Wk_p, "Wv": Wv_h,
            "cosA": np.ascontiguousarray(cosT[:, rows]),
            "sinA": np.ascontiguousarray(sinT[:, rows]),
        })
    res_a = run_bass_kernel_spmd(nca, in_a, list(range(N_CORES)))

    kr_full = [np.concatenate([res_a.results[b * 4 + s]["krA"] for s in range(4)], axis=1)
               for b in range(B)]   # [C, T] fp16
    v_full = [np.concatenate([res_a.results[b * 4 + s]["vA"] for s in range(4)], axis=0)
              for b in range(B)]    # [T, NB*C] fp16

    # ---- phase B: q + attention ----
    ncb = build_phase_b()
    in_b = []
    for core in range(N_CORES):
        b, j = divmod(core, 4)
        tcols = np.concatenate([np.arange(BLK * tb, BLK * (tb + 1))
                                for tb in _core_blocks(j)])
        in_b.append({
            "aQ": np.ascontiguousarray(a[b].T[:, tcols]).astype(NPF16),
            "Wq": Wq_p,
            "cosB": np.ascontiguousarray(cosT[:, tcols]),
            "sinB": np.ascontiguousarray(sinT[:, tcols]),
            "krB": kr_full[b],
            "vB": v_full[b],
            "Wo": Wo_h,
            "mskd": _masks(j),
        })
    res_b = run_bass_kernel_spmd(ncb, in_b, list(range(N_CORES)))

    outf = np.zeros((B, T, C), np.float32)
    for core in range(N_CORES):
        b, j = divmod(core, 4)
        o = res_b.results[core]["o"]
        for sl, tb in enumerate(_core_blocks(j)):
            outf[b, BLK * tb:BLK * (tb + 1)] = o[sl * 128:(sl + 1) * 128]
    return outf


# revision 53
# speedup vs baseline: 1.0075x; 1.0075x over previous
"""Trainium2 Bass kernel for nn_Attention_85710367359290 (sparse branch-routed attention).

Semantics (validated vs reference offline, rel err ~0.011):
  q = rope(a @ Wq) per branch (NB=4), k = rope(x @ Wk), v = a @ Wv per branch
  att[b,n,t,s] = q.k/sqrt(C);  amax = max_n exp(att);  p = amax (no max-sub, |att|<~8)
  routing: cmb_n = p * (exp(att_n) >= amax) on causal positions
  y = sum_n cmb_n @ v_n;  Z = sum_s p;  out = (y/Z) @ Wo

Two-phase SPMD over 8 cores (no collectives; host reshuffles between phases).
All tensor data fp16 (PSUM accum f32).

Phase A: k-proj+rope and v-proj, core (b,s4) owns a 512-row s-slice of batch b.
Phase B: q-proj+rope + attention. Core (b,j) owns 4 t-blocks of 128 rows:
  tb = {15-j, 11-j, 7-j, 3-j} assigned to slots 0..3 with fixed per-slot
  s-chunk capacities (16,12,8,4) >= needs (16-j,12-j,8-j,4-j): a uniform
  40-unit program (si-major so each v s-chunk is loaded once).
  Padded units are neutralized by all-zero masks (data).

Unit (si, slot): att[s128, (n4,t128)] = kr_si^T q_slot (4 matmuls ap512 fp16);
  e = exp(att-4) fp16 (Act); amax = max_n e (2 TT DVE); p_m = amax*msk (DVE);
  mb = e>=amax bcast (Pool); cmb = mb*p_m bcast (DVE); Z[:,slot] += colsum
  (PE ones-matmul); y[slot] += v_n^T cmb_n (16 matmuls ap128).
  Epilogue: o = (y/Z)^T @ Wo.

PSUM matmul start=True zeroes the whole bank: only the chronologically first
matmul per bank sets it. DMAs are batched into large strided transfers
(SBUF-resident kr/v/masks) to keep the SP sequencer off the critical path.
"""

import numpy as np
import ml_dtypes

import concourse.bass as bass
import concourse.mybir as mybir
import concourse.tile as tile
from concourse import bacc
from concourse.bass_utils import run_bass_kernel_spmd

F32 = mybir.dt.float32
F16 = mybir.dt.float16
ALU = mybir.AluOpType
ACTF = mybir.ActivationFunctionType
NPF16 = np.float16

B, T, C, NB = 2, 2048, 512, 4
SC = 128          # s-chunk (PSUM partition dim)
BLK = 128         # t-block
NSI = T // SC     # 16
CAPS = (16, 12, 8, 4)
# interleave slots proportionally so consecutive units hit different slots
# (v is SBUF-resident, so any order is legal for the accumulations)
UNITS = sorted(
    [(si, sl) for si in range(NSI) for sl in range(4) if si < CAPS[sl]],
    key=lambda u: ((u[0] + 0.5) / CAPS[u[1]], u[1]))
NU = len(UNITS)   # 40
N_CORES = 8
EXP_BIAS = -4.0

_cache = {}


def build_phase_a():
    if "a" in _cache:
        return _cache["a"]
    nc = bacc.Bacc("TRN2", target_bir_lowering=False, debug=False)

    def din(name, shape, dt):
        return nc.dram_tensor(name, shape, dt, kind="ExternalInput").ap()

    xT = din("xT", [C, 512], F16)       # x[b].T cols of this core's s-slice
    aTv = din("aTv", [C, 512], F16)     # a[b].T same cols (for v)
    Wk = din("Wk", [C, C], F16)         # split-permuted, pre-scaled 1/sqrt(C)
    Wv = din("Wv", [C, NB * C], F16)
    cosA = din("cosA", [C // 2, 512], F16)
    sinA = din("sinA", [C // 2, 512], F16)
    krA = nc.dram_tensor("krA", [C, 512], F16, kind="ExternalOutput").ap()
    vA = nc.dram_tensor("vA", [512, NB * C], F16, kind="ExternalOutput").ap()

    def v4(ap):  # DRAM [512, W] -> [128, (4, W)] chunk-major
        return ap.rearrange("(c p) w -> p c w", p=128)

    with tile.TileContext(nc) as tc:
        with (
            tc.tile_pool(name="pa", bufs=1) as pa,
            tc.tile_pool(name="pat", bufs=4) as pat,
            tc.tile_pool(name="pap", bufs=6, space="PSUM") as pps,
        ):
            xB = pa.tile([128, 4 * 512], F16, tag="xB", name="xB")
            aB = pa.tile([128, 4 * 512], F16, tag="aB", name="aB")
            WkB = pa.tile([128, 4 * C], F16, tag="WkB", name="WkB")
            WvB = pa.tile([128, 4 * NB * C], F16, tag="WvB", name="WvB")
            csB = pa.tile([128, 2 * 512], F16, tag="csB", name="csB")
            snB = pa.tile([128, 2 * 512], F16, tag="snB", name="snB")
            WvBv = WvB.rearrange("p (c n w) -> p c n w", c=4, n=NB)
            Wvv = v4(Wv).rearrange("p c (n w) -> p c n w", n=NB)
            for c in range(4):
                nc.sync.dma_start(
                    out=xB.rearrange("p (c w) -> p c w", c=4)[:, c:c + 1, :],
                    in_=v4(xT)[:, c:c + 1, :])
                nc.sync.dma_start(
                    out=WkB.rearrange("p (c w) -> p c w", c=4)[:, c:c + 1, :],
                    in_=v4(Wk)[:, c:c + 1, :])
            nc.sync.dma_start(out=csB.rearrange("p (c w) -> p c w", c=2),
                              in_=cosA.rearrange("(c p) w -> p c w", p=128))
            nc.sync.dma_start(out=snB.rearrange("p (c w) -> p c w", c=2),
                              in_=sinA.rearrange("(c p) w -> p c w", p=128))
            nc.sync.dma_start(out=aB.rearrange("p (c w) -> p c w", c=4), in_=v4(aTv))
            for n in range(NB):
                nc.sync.dma_start(out=WvBv[:, :, n, :], in_=Wvv[:, :, n, :])
            xTt = [xB[:, i * 512:(i + 1) * 512] for i in range(4)]
            aTt = [aB[:, i * 512:(i + 1) * 512] for i in range(4)]
            WkT = [WkB[:, i * C:(i + 1) * C] for i in range(4)]
            WvT = [WvB[:, i * NB * C:(i + 1) * NB * C] for i in range(4)]
            cst = [csB[:, i * 512:(i + 1) * 512] for i in range(2)]
            snt = [snB[:, i * 512:(i + 1) * 512] for i in range(2)]

            # ---- k proj -> fp16 copy -> rope -> krA ----
            kp16 = [pa.tile([128, 512], F16, tag=f"kp{i}", name=f"kp{i}") for i in range(4)]
            for m in range(4):
                ps = pps.tile([128, 512], F32, tag="pps", name="pps")
                for Kc in range(4):
                    nc.tensor.matmul(ps, WkT[Kc][:, m * 128:(m + 1) * 128], xTt[Kc],
                                     start=(Kc == 0), stop=(Kc == 3))
                nc.scalar.copy(out=kp16[m], in_=ps)
            for h in range(2):
                t1 = pat.tile([128, 512], F16, tag="t1", name="t1")
                t2 = pat.tile([128, 512], F16, tag="t2", name="t2")
                kr = pat.tile([128, 512], F16, tag="kr", name="kr")
                nc.vector.tensor_mul(t1, kp16[h], cst[h])
                nc.vector.tensor_mul(t2, kp16[2 + h], snt[h])
                nc.vector.tensor_sub(kr, t1, t2)
                nc.sync.dma_start(out=krA[h * 128:(h + 1) * 128, :], in_=kr)
                t3 = pat.tile([128, 512], F16, tag="t3", name="t3")
                t4 = pat.tile([128, 512], F16, tag="t4", name="t4")
                kr2 = pat.tile([128, 512], F16, tag="kr2", name="kr2")
                nc.vector.tensor_mul(t3, kp16[h], snt[h])
                nc.vector.tensor_mul(t4, kp16[2 + h], cst[h])
                nc.vector.tensor_add(kr2, t3, t4)
                nc.sync.dma_start(out=krA[(2 + h) * 128:(3 + h) * 128, :], in_=kr2)

            # ---- v proj: vA[s-slice, (n,c)]; nb-outer so Wv(nb0) starts early ----
            vsb = [pa.tile([128, NB * 512], F16, tag=f"vsb{i}", name=f"vsb{i}")
                   for i in range(4)]
            for nb in range(4):
                for sc in range(4):
                    ps = pps.tile([128, 512], F32, tag="pps", name="pps")
                    for Kc in range(4):
                        nc.tensor.matmul(ps, aTt[Kc][:, sc * 128:(sc + 1) * 128],
                                         WvT[Kc][:, nb * 512:(nb + 1) * 512],
                                         start=(Kc == 0), stop=(Kc == 3))
                    dst = vsb[sc][:, nb * 512:(nb + 1) * 512]
                    if (nb + sc) % 2 == 0:
                        nc.scalar.copy(out=dst, in_=ps)
                    else:
                        nc.vector.tensor_scalar_mul(dst, ps, 1.0)
                if nb % 2 == 1:  # ship completed halves early
                    for sc in range(4):
                        nc.sync.dma_start(
                            out=vA[sc * 128:(sc + 1) * 128,
                                   (nb - 1) * 512:(nb + 1) * 512],
                            in_=vsb[sc][:, (nb - 1) * 512:(nb + 1) * 512])
    nc.compile()
    _cache["a"] = nc
    return nc


def build_phase_b():
    if "b" in _cache:
        return _cache["b"]
    nc = bacc.Bacc("TRN2", target_bir_lowering=False, debug=False)

    def din(name, shape, dt):
        return nc.dram_tensor(name, shape, dt, kind="ExternalInput").ap()

    aQ = din("aQ", [C, 512], F16)       # a[b].T cols = 4 t-blocks (slot order)
    Wq = din("Wq", [C, NB * C], F16)    # split-permuted
    cosB = din("cosB", [C // 2, 512], F16)
    sinB = din("sinB", [C // 2, 512], F16)
    krB = din("krB", [C, T], F16)
    vB = din("vB", [T, NB * C], F16)
    WoD = din("Wo", [C, C], F16)
    mskd = din("mskd", [NU, SC, BLK], mybir.dt.float8e4)
    out = nc.dram_tensor("o", [512, C], F32, kind="ExternalOutput").ap()

    with tile.TileContext(nc) as tc:
        with (
            tc.tile_pool(name="persist", bufs=1) as pp,
            tc.tile_pool(name="qtmp", bufs=4) as qtp,
            tc.tile_pool(name="ew", bufs=5) as ew,
            tc.tile_pool(name="ep", bufs=2) as epi,
        ):
            aQb = pp.tile([128, 4 * 512], F16, tag="aQb", name="aQb")
            WqB = pp.tile([128, 4 * NB * C], F16, tag="WqB", name="WqB")
            csB = pp.tile([128, 2 * 512], F16, tag="csB", name="csB")
            snB = pp.tile([128, 2 * 512], F16, tag="snB", name="snB")
            krBt = pp.tile([128, 4 * T], F16, tag="krBt", name="krBt")
            vBt = pp.tile([128, NSI * NB * C], F16, tag="vBt", name="vBt")
            WoB = pp.tile([128, 4 * C], F16, tag="WoB", name="WoB")
            mskB = pp.tile([128, NU * BLK], mybir.dt.float8e4, tag="mskB", name="mskB")
            # qrT[m]: partition = c' in chunk m; free = (n 4, t 512)
            qrT = [pp.tile([128, NB * 512], F16, tag=f"qr{i}", name=f"qr{i}") for i in range(4)]
            ones = pp.tile([128, 1], F16, tag="ones", name="ones")
            ebias = pp.tile([128, 1], F32, tag="ebias", name="ebias")
            nc.vector.memset(ones, 1.0)
            nc.vector.memset(ebias, EXP_BIAS)

            def v4(ap, p=128):  # DRAM [4*p, W] -> [p, (4, W)]
                return ap.rearrange("(c p) w -> p c w", p=p)

            # deliver in consumption order: q-path, kr si-quarters and v per-si
            # chunks interleaved just-in-time; masks in halves; Wo last
            aQv = aQb.rearrange("p (c w) -> p c w", c=4)
            aQs = v4(aQ)
            WqBv = WqB.rearrange("p (c n w) -> p c n w", c=4, n=NB)
            Wqv = v4(Wq).rearrange("p c (n w) -> p c n w", n=NB)
            krv = krBt.rearrange("p (c g w) -> p c g w", c=4, g=4)
            krs = v4(krB).rearrange("p c (g w) -> p c g w", g=4)
            vBv = vB.rearrange("(g p) w -> p g w", p=128)  # [128, 16, 2048]
            vtv = vBt.rearrange("p (g w) -> p g w", g=NSI)
            mskv = mskB.rearrange("p (u w) -> p u w", u=NU)
            msks = mskd.rearrange("u p w -> p u w")

            def vdma(si):
                nc.sync.dma_start(out=vtv[:, si:si + 1, :], in_=vBv[:, si:si + 1, :])

            nc.sync.dma_start(out=aQv[:, 0:2, :], in_=aQs[:, 0:2, :])
            nc.sync.dma_start(out=WqBv[:, 0:2, 0, :], in_=Wqv[:, 0:2, 0, :])
            nc.sync.dma_start(out=aQv[:, 2:4, :], in_=aQs[:, 2:4, :])
            nc.sync.dma_start(out=WqBv[:, 2:4, 0, :], in_=Wqv[:, 2:4, 0, :])
            nc.sync.dma_start(out=krv[:, :, 0, :], in_=krs[:, :, 0, :])
            nc.sync.dma_start(out=WqBv[:, :, 1, :], in_=Wqv[:, :, 1, :])
            vdma(0)
            nc.sync.dma_start(out=csB.rearrange("p (c w) -> p c w", c=2), in_=v4(cosB))
            nc.sync.dma_start(out=snB.rearrange("p (c w) -> p c w", c=2), in_=v4(sinB))
            nc.sync.dma_start(out=mskv[:, 0:20, :], in_=msks[:, 0:20, :])
            vdma(1)
            nc.sync.dma_start(out=WqBv[:, :, 2, :], in_=Wqv[:, :, 2, :])
            vdma(2)
            vdma(3)
            nc.sync.dma_start(out=WqBv[:, :, 3, :], in_=Wqv[:, :, 3, :])
            nc.sync.dma_start(out=krv[:, :, 1, :], in_=krs[:, :, 1, :])
            vdma(4)
            vdma(5)
            nc.sync.dma_start(out=krv[:, :, 2, :], in_=krs[:, :, 2, :])
            vdma(6)
            vdma(7)
            nc.sync.dma_start(out=mskv[:, 20:NU, :], in_=msks[:, 20:NU, :])
            vdma(8)
            vdma(9)
            nc.sync.dma_start(out=krv[:, :, 3, :], in_=krs[:, :, 3, :])
            for si in range(10, NSI):
                vdma(si)
            nc.sync.dma_start(out=WoB.rearrange("p (c w) -> p c w", c=4), in_=v4(WoD))
            aQt = [aQb[:, i * 512:(i + 1) * 512] for i in range(4)]
            WqT = [WqB[:, i * NB * C:(i + 1) * NB * C] for i in range(4)]
            cst = [csB[:, i * 512:(i + 1) * 512] for i in range(2)]
            snt = [snB[:, i * 512:(i + 1) * 512] for i in range(2)]
            krT = [krBt[:, i * T:(i + 1) * T] for i in range(4)]
            WoT = [WoB[:, i * C:(i + 1) * C] for i in range(4)]

            with tc.tile_pool(name="qpp", bufs=8, space="PSUM") as qpp:
                # ---- q proj + rope (all 4 blocks at once) ----
                for n in range(NB):
                    qp16 = [qtp.tile([128, 512], F16, tag=f"qp{m}", name=f"qp{m}")
                            for m in range(4)]
                    for m in range(4):
                        ps = qpp.tile([128, 512], F32, tag="qps", name="qps")
                        for Kc in range(4):
                            nc.tensor.matmul(
                                ps, WqT[Kc][:, (4 * n + m) * 128:(4 * n + m + 1) * 128],
                                aQt[Kc], start=(Kc == 0), stop=(Kc == 3))
                        nc.scalar.copy(out=qp16[m], in_=ps)
                    for h in range(2):
                        eng = nc.vector if h == 0 else nc.gpsimd
                        t1 = qtp.tile([128, 512], F16, tag="qt1", name="qt1")
                        t2 = qtp.tile([128, 512], F16, tag="qt2", name="qt2")
                        eng.tensor_mul(t1, qp16[h], cst[h])
                        eng.tensor_mul(t2, qp16[2 + h], snt[h])
                        nc.vector.tensor_sub(qrT[h][:, n * 512:(n + 1) * 512], t1, t2)
                        t3 = qtp.tile([128, 512], F16, tag="qt3", name="qt3")
                        t4 = qtp.tile([128, 512], F16, tag="qt4", name="qt4")
                        eng.tensor_mul(t3, qp16[h], snt[h])
                        eng.tensor_mul(t4, qp16[2 + h], cst[h])
                        nc.vector.tensor_add(qrT[2 + h][:, n * 512:(n + 1) * 512], t3, t4)

            with tc.tile_pool(name="accp", bufs=1, space="PSUM") as acc:
                yT = [acc.tile([128, 512], F32, tag=f"yT{i}", name=f"yT{i}")
                      for i in range(4)]
                Zp = acc.tile([128, 4], F32, tag="Zp", name="Zp")
                qv = [qrT[Kc].rearrange("p (n t) -> p n t", n=NB) for Kc in range(4)]
                DEPTH = 2  # PE software-pipeline depth: att(u+2) before pv(u)

                with tc.tile_pool(name="attp", bufs=3, space="PSUM") as app:
                    first_u = {sl: min(i for i, u in enumerate(UNITS) if u[1] == sl)
                               for sl in range(4)}
                    last_u = {sl: max(i for i, u in enumerate(UNITS) if u[1] == sl)
                              for sl in range(4)}

                    def emit_epilogue(sl):
                        # o = (y/Z)^T @ Wo; reuse the slot's dead yT bank
                        yb = epi.tile([128, 512], F16, tag="yb", name="yb")
                        nc.scalar.copy(out=yb, in_=yT[sl])
                        zr = epi.tile([128, 1], F32, tag="zr", name="zr")
                        nc.vector.reciprocal(zr, Zp[:, sl:sl + 1])
                        ops = acc.tile([128, 512], F32, tag=f"yT{sl}", name=f"ops{sl}")
                        for Mc in range(4):
                            nc.tensor.matmul(ops, yb[:, Mc * 128:(Mc + 1) * 128],
                                             WoT[Mc], start=(Mc == 0), stop=(Mc == 3))
                        osb = epi.tile([128, 512], F32, tag="osb", name="osb")
                        nc.vector.tensor_scalar_mul(osb, ops, zr[:, 0:1])
                        nc.sync.dma_start(out=out[sl * 128:(sl + 1) * 128, :], in_=osb)

                    def emit_zpv(ui, si, sl, p_m, cmb):
                        start = ui == first_u[sl]
                        stop = ui == last_u[sl]
                        vt = vBt[:, si * NB * C:(si + 1) * NB * C]
                        # start=True zeroes the whole PSUM bank -> only first
                        nc.tensor.matmul(Zp[:, sl:sl + 1], p_m, ones,
                                         start=(ui == 0), stop=stop)
                        for br in range(NB):
                            for Mc in range(4):
                                nc.tensor.matmul(
                                    yT[sl][:, Mc * 128:(Mc + 1) * 128],
                                    vt[:, br * 512 + Mc * 128:br * 512 + (Mc + 1) * 128],
                                    cmb[:, br * 128:(br + 1) * 128],
                                    start=(start and br == 0 and Mc == 0),
                                    stop=(stop and br == 3))
                        if stop:
                            emit_epilogue(sl)

                    pending = []
                    for ui, (si, sl) in enumerate(UNITS):
                        msk = mskB[:, ui * BLK:(ui + 1) * BLK]
                        att = app.tile([128, NB * BLK], F32, tag="att", name="att")
                        attv = att.rearrange("p (n t) -> p n t", n=NB)
                        for bp in range(2):
                            for Kc in range(4):
                                nc.tensor.matmul(
                                    attv[:, bp * 2:(bp + 1) * 2],
                                    krT[Kc][:, si * 128:(si + 1) * 128],
                                    qv[Kc][:, bp * 2:(bp + 1) * 2,
                                           sl * 128:(sl + 1) * 128],
                                    start=(bp == 0 and Kc == 0), stop=(Kc == 3))
                        e = ew.tile([128, NB * BLK], F16, tag="e", name="e")
                        nc.scalar.activation(out=e, in_=att, func=ACTF.Exp,
                                             bias=ebias[:, 0:1])
                        m1 = ew.tile([128, 256], F16, tag="m1", name="m1")
                        amax = ew.tile([128, BLK], F16, tag="amax", name="amax")
                        p_m = ew.tile([128, BLK], F16, tag="p_m", name="p_m")
                        nc.vector.tensor_max(m1, e[:, 0:256], e[:, 256:512])
                        nc.vector.tensor_max(amax, m1[:, 0:128], m1[:, 128:256])
                        nc.vector.tensor_mul(p_m, amax, msk)
                        amax_b = amax.rearrange("p (a t) -> p a t", a=1).to_broadcast(
                            [128, NB, BLK])
                        pm_b = p_m.rearrange("p (a t) -> p a t", a=1).to_broadcast(
                            [128, NB, BLK])
                        mb = ew.tile([128, NB * BLK], F16, tag="mb", name="mb")
                        cmb = ew.tile([128, NB * BLK], F16, tag="cmb", name="cmb")
                        ev = e.rearrange("p (n t) -> p n t", n=NB)
                        nc.vector.tensor_tensor(
                            out=mb.rearrange("p (n t) -> p n t", n=NB),
                            in0=ev, in1=amax_b, op=ALU.is_ge)
                        nc.vector.tensor_mul(
                            cmb.rearrange("p (n t) -> p n t", n=NB)[:, 0:2],
                            mb.rearrange("p (n t) -> p n t", n=NB)[:, 0:2],
                            pm_b[:, 0:2])
                        nc.gpsimd.tensor_mul(
                            cmb.rearrange("p (n t) -> p n t", n=NB)[:, 2:4],
                            mb.rearrange("p (n t) -> p n t", n=NB)[:, 2:4],
                            pm_b[:, 2:4])
                        pending.append((ui, si, sl, p_m, cmb))
                        if len(pending) > DEPTH:
                            emit_zpv(*pending.pop(0))
                    while pending:
                        emit_zpv(*pending.pop(0))
    nc.compile()
    _cache["b"] = nc
    return nc


def _host_prep(Wq, Wk, Wv, Wo, cos, sin):
    split_idx = np.r_[0:C:2, 1:C:2]
    Wq_p = np.ascontiguousarray(
        Wq.reshape(C, NB, C)[:, :, split_idx].reshape(C, NB * C)).astype(NPF16)
    Wk_p = (Wk[:, split_idx] * np.float32(1.0 / np.sqrt(C))).astype(NPF16)
    Wv_h = Wv.astype(NPF16)
    Wo_h = Wo.astype(NPF16)
    cosT = np.ascontiguousarray(cos[:T].T).astype(NPF16)  # [C/2, T]
    sinT = np.ascontiguousarray(sin[:T].T).astype(NPF16)
    return Wq_p, Wk_p, Wv_h, Wo_h, cosT, sinT


def _core_blocks(j):
    return [15 - j, 11 - j, 7 - j, 3 - j]


def _masks(j):
    m = np.zeros((NU, SC, BLK), np.float32)
    tbs = _core_blocks(j)
    tt = np.arange(BLK)[None, :]
    ss = np.arange(SC)[:, None]
    for ui, (si, sl) in enumerate(UNITS):
        t0 = BLK * tbs[sl]
        m[ui] = (t0 + tt) >= (SC * si + ss)
    return m.astype(ml_dtypes.float8_e4m3)


def kernel(a, x, Wq, Wk, Wv, Wo, cos, sin):
    a = np.asarray(a, np.float32)
    x = np.asarray(x, np.float32)
    Wq_p, Wk_p, Wv_h, Wo_h, cosT, sinT = _host_prep(
        np.asarray(Wq, np.float32), np.asarray(Wk, np.float32),
        np.asarray(Wv, np.float32), np.asarray(Wo, np.float32),
        np.asarray(cos, np.float32), np.asarray(sin, np.float32))

    # ---- phase A: k + v ----
    nca = build_phase_a()
    in_a = []
    for core in range(N_CORES):
        b, s4 = divmod(core, 4)
        rows = slice(512 * s4, 512 * (s4 + 1))
        in_a.append({
            "xT": np.ascontiguousarray(x[b].T[:, rows]).astype(NPF16),
            "aTv": np.ascontiguousarray(a[b].T[:, rows]).astype(NPF16),
            "Wk": Wk_p, "Wv": Wv_h,
            "cosA": np.ascontiguousarray(cosT[:, rows]),
            "sinA": np.ascontiguousarray(sinT[:, rows]),
        })
    res_a = run_bass_kernel_spmd(nca, in_a, list(range(N_CORES)))

    kr_full = [np.concatenate([res_a.results[b * 4 + s]["krA"] for s in range(4)], axis=1)
               for b in range(B)]   # [C, T] fp16
    v_full = [np.concatenate([res_a.results[b * 4 + s]["vA"] for s in range(4)], axis=0)
              for b in range(B)]    # [T, NB*C] fp16

    # ---- phase B: q + attention ----
    ncb = build_phase_b()
    in_b = []
    for core in range(N_CORES):
        b, j = divmod(core, 4)
        tcols = np.concatenate([np.arange(BLK * tb, BLK * (tb + 1))
                                for tb in _core_blocks(j)])
        in_b.append({
            "aQ": np.ascontiguousarray(a[b].T[:, tcols]).astype(NPF16),
            "Wq": Wq_p,
            "cosB": np.ascontiguousarray(cosT[:, tcols]),
            "sinB": np.ascontiguousarray(sinT[:, tcols]),
            "krB": kr_full[b],
            "vB": v_full[b],
            "Wo": Wo_h,
            "mskd": _masks(j),
        })
    res_b = run_bass_kernel_spmd(ncb, in_b, list(range(N_CORES)))

    outf = np.zeros((B, T, C), np.float32)
    for core in range(N_CORES):
        b, j = divmod(core, 4)
        o = res_b.results[core]["o"]
        for sl, tb in enumerate(_core_blocks(j)):
            outf[b, BLK * tb:BLK * (tb + 1)] = o[sl * 128:(sl + 1) * 128]
    return outf


# revision 54
# speedup vs baseline: 1.0188x; 1.0112x over previous
"""Trainium2 Bass kernel for nn_Attention_85710367359290 (sparse branch-routed attention).

Semantics (validated vs reference offline, rel err ~0.011):
  q = rope(a @ Wq) per branch (NB=4), k = rope(x @ Wk), v = a @ Wv per branch
  att[b,n,t,s] = q.k/sqrt(C);  amax = max_n exp(att);  p = amax (no max-sub, |att|<~8)
  routing: cmb_n = p * (exp(att_n) >= amax) on causal positions
  y = sum_n cmb_n @ v_n;  Z = sum_s p;  out = (y/Z) @ Wo

Two-phase SPMD over 8 cores (no collectives; host reshuffles between phases).
All tensor data fp16 (PSUM accum f32).

Phase A: k-proj+rope and v-proj, core (b,s4) owns a 512-row s-slice of batch b.
Phase B: q-proj+rope + attention. Core (b,j) owns 4 t-blocks of 128 rows:
  tb = {15-j, 11-j, 7-j, 3-j} assigned to slots 0..3 with fixed per-slot
  s-chunk capacities (16,12,8,4) >= needs (16-j,12-j,8-j,4-j): a uniform
  40-unit program (si-major so each v s-chunk is loaded once).
  Padded units are neutralized by all-zero masks (data).

Unit (si, slot): att[s128, (n4,t128)] = kr_si^T q_slot (4 matmuls ap512 fp16);
  e = exp(att-4) fp16 (Act); amax = max_n e (2 TT DVE); p_m = amax*msk (DVE);
  mb = e>=amax bcast (Pool); cmb = mb*p_m bcast (DVE); Z[:,slot] += colsum
  (PE ones-matmul); y[slot] += v_n^T cmb_n (16 matmuls ap128).
  Epilogue: o = (y/Z)^T @ Wo.

PSUM matmul start=True zeroes the whole bank: only the chronologically first
matmul per bank sets it. DMAs are batched into large strided transfers
(SBUF-resident kr/v/masks) to keep the SP sequencer off the critical path.
"""

import numpy as np
import ml_dtypes

import concourse.bass as bass
import concourse.mybir as mybir
import concourse.tile as tile
from concourse import bacc
from concourse.bass_utils import run_bass_kernel_spmd

F32 = mybir.dt.float32
F16 = mybir.dt.float16
ALU = mybir.AluOpType
ACTF = mybir.ActivationFunctionType
NPF16 = np.float16

B, T, C, NB = 2, 2048, 512, 4
SC = 128          # s-chunk (PSUM partition dim)
BLK = 128         # t-block
NSI = T // SC     # 16
CAPS = (16, 12, 8, 4)
# interleave slots proportionally so consecutive units hit different slots
# (v is SBUF-resident, so any order is legal for the accumulations)
UNITS = sorted(
    [(si, sl) for si in range(NSI) for sl in range(4) if si < CAPS[sl]],
    key=lambda u: ((u[0] + 0.5) / CAPS[u[1]], u[1]))
NU = len(UNITS)   # 40
N_CORES = 8
EXP_BIAS = -4.0

_cache = {}


def build_phase_a():
    if "a" in _cache:
        return _cache["a"]
    nc = bacc.Bacc("TRN2", target_bir_lowering=False, debug=False)

    def din(name, shape, dt):
        return nc.dram_tensor(name, shape, dt, kind="ExternalInput").ap()

    xT = din("xT", [C, 512], F16)       # x[b].T cols of this core's s-slice
    aTv = din("aTv", [C, 512], F16)     # a[b].T same cols (for v)
    Wk = din("Wk", [C, C], F16)         # split-permuted, pre-scaled 1/sqrt(C)
    Wv = din("Wv", [C, NB * C], F16)
    cosA = din("cosA", [C // 2, 512], F16)
    sinA = din("sinA", [C // 2, 512], F16)
    krA = nc.dram_tensor("krA", [C, 512], F16, kind="ExternalOutput").ap()
    vA = nc.dram_tensor("vA", [512, NB * C], F16, kind="ExternalOutput").ap()

    def v4(ap):  # DRAM [512, W] -> [128, (4, W)] chunk-major
        return ap.rearrange("(c p) w -> p c w", p=128)

    with tile.TileContext(nc) as tc:
        with (
            tc.tile_pool(name="pa", bufs=1) as pa,
            tc.tile_pool(name="pat", bufs=4) as pat,
            tc.tile_pool(name="pap", bufs=6, space="PSUM") as pps,
        ):
            xB = pa.tile([128, 4 * 512], F16, tag="xB", name="xB")
            aB = pa.tile([128, 4 * 512], F16, tag="aB", name="aB")
            WkB = pa.tile([128, 4 * C], F16, tag="WkB", name="WkB")
            WvB = pa.tile([128, 4 * NB * C], F16, tag="WvB", name="WvB")
            csB = pa.tile([128, 2 * 512], F16, tag="csB", name="csB")
            snB = pa.tile([128, 2 * 512], F16, tag="snB", name="snB")
            WvBv = WvB.rearrange("p (c n w) -> p c n w", c=4, n=NB)
            Wvv = v4(Wv).rearrange("p c (n w) -> p c n w", n=NB)
            for c in range(4):
                nc.sync.dma_start(
                    out=xB.rearrange("p (c w) -> p c w", c=4)[:, c:c + 1, :],
                    in_=v4(xT)[:, c:c + 1, :])
                nc.scalar.dma_start(
                    out=WkB.rearrange("p (c w) -> p c w", c=4)[:, c:c + 1, :],
                    in_=v4(Wk)[:, c:c + 1, :])
            nc.sync.dma_start(out=csB.rearrange("p (c w) -> p c w", c=2),
                              in_=cosA.rearrange("(c p) w -> p c w", p=128))
            nc.sync.dma_start(out=snB.rearrange("p (c w) -> p c w", c=2),
                              in_=sinA.rearrange("(c p) w -> p c w", p=128))
            nc.sync.dma_start(out=aB.rearrange("p (c w) -> p c w", c=4), in_=v4(aTv))
            for n in range(NB):
                nc.sync.dma_start(out=WvBv[:, :, n, :], in_=Wvv[:, :, n, :])
            xTt = [xB[:, i * 512:(i + 1) * 512] for i in range(4)]
            aTt = [aB[:, i * 512:(i + 1) * 512] for i in range(4)]
            WkT = [WkB[:, i * C:(i + 1) * C] for i in range(4)]
            WvT = [WvB[:, i * NB * C:(i + 1) * NB * C] for i in range(4)]
            cst = [csB[:, i * 512:(i + 1) * 512] for i in range(2)]
            snt = [snB[:, i * 512:(i + 1) * 512] for i in range(2)]

            # ---- k proj -> fp16 copy -> rope -> krA ----
            kp16 = [pa.tile([128, 512], F16, tag=f"kp{i}", name=f"kp{i}") for i in range(4)]
            for m in range(4):
                ps = pps.tile([128, 512], F32, tag="pps", name="pps")
                for Kc in range(4):
                    nc.tensor.matmul(ps, WkT[Kc][:, m * 128:(m + 1) * 128], xTt[Kc],
                                     start=(Kc == 0), stop=(Kc == 3))
                nc.scalar.copy(out=kp16[m], in_=ps)
            for h in range(2):
                t1 = pat.tile([128, 512], F16, tag="t1", name="t1")
                t2 = pat.tile([128, 512], F16, tag="t2", name="t2")
                kr = pat.tile([128, 512], F16, tag="kr", name="kr")
                nc.vector.tensor_mul(t1, kp16[h], cst[h])
                nc.vector.tensor_mul(t2, kp16[2 + h], snt[h])
                nc.vector.tensor_sub(kr, t1, t2)
                nc.sync.dma_start(out=krA[h * 128:(h + 1) * 128, :], in_=kr)
                t3 = pat.tile([128, 512], F16, tag="t3", name="t3")
                t4 = pat.tile([128, 512], F16, tag="t4", name="t4")
                kr2 = pat.tile([128, 512], F16, tag="kr2", name="kr2")
                nc.vector.tensor_mul(t3, kp16[h], snt[h])
                nc.vector.tensor_mul(t4, kp16[2 + h], cst[h])
                nc.vector.tensor_add(kr2, t3, t4)
                nc.sync.dma_start(out=krA[(2 + h) * 128:(3 + h) * 128, :], in_=kr2)

            # ---- v proj: vA[s-slice, (n,c)]; nb-outer so Wv(nb0) starts early ----
            vsb = [pa.tile([128, NB * 512], F16, tag=f"vsb{i}", name=f"vsb{i}")
                   for i in range(4)]
            for nb in range(4):
                for sc in range(4):
                    ps = pps.tile([128, 512], F32, tag="pps", name="pps")
                    for Kc in range(4):
                        nc.tensor.matmul(ps, aTt[Kc][:, sc * 128:(sc + 1) * 128],
                                         WvT[Kc][:, nb * 512:(nb + 1) * 512],
                                         start=(Kc == 0), stop=(Kc == 3))
                    dst = vsb[sc][:, nb * 512:(nb + 1) * 512]
                    if (nb + sc) % 2 == 0:
                        nc.scalar.copy(out=dst, in_=ps)
                    else:
                        nc.vector.tensor_scalar_mul(dst, ps, 1.0)
                if nb % 2 == 1:  # ship completed halves early, 2 queues
                    for sc in range(4):
                        eng = nc.sync if sc % 2 == 0 else nc.scalar
                        eng.dma_start(
                            out=vA[sc * 128:(sc + 1) * 128,
                                   (nb - 1) * 512:(nb + 1) * 512],
                            in_=vsb[sc][:, (nb - 1) * 512:(nb + 1) * 512])
    nc.compile()
    _cache["a"] = nc
    return nc


def build_phase_b():
    if "b" in _cache:
        return _cache["b"]
    nc = bacc.Bacc("TRN2", target_bir_lowering=False, debug=False)

    def din(name, shape, dt):
        return nc.dram_tensor(name, shape, dt, kind="ExternalInput").ap()

    aQ = din("aQ", [C, 512], F16)       # a[b].T cols = 4 t-blocks (slot order)
    Wq = din("Wq", [C, NB * C], F16)    # split-permuted
    cosB = din("cosB", [C // 2, 512], F16)
    sinB = din("sinB", [C // 2, 512], F16)
    krB = din("krB", [C, T], F16)
    vB = din("vB", [T, NB * C], F16)
    WoD = din("Wo", [C, C], F16)
    mskd = din("mskd", [NU, SC, BLK], mybir.dt.float8e4)
    out = nc.dram_tensor("o", [512, C], F32, kind="ExternalOutput").ap()

    with tile.TileContext(nc) as tc:
        with (
            tc.tile_pool(name="persist", bufs=1) as pp,
            tc.tile_pool(name="qtmp", bufs=4) as qtp,
            tc.tile_pool(name="ew", bufs=5) as ew,
            tc.tile_pool(name="ep", bufs=2) as epi,
        ):
            aQb = pp.tile([128, 4 * 512], F16, tag="aQb", name="aQb")
            WqB = pp.tile([128, 4 * NB * C], F16, tag="WqB", name="WqB")
            csB = pp.tile([128, 2 * 512], F16, tag="csB", name="csB")
            snB = pp.tile([128, 2 * 512], F16, tag="snB", name="snB")
            krBt = pp.tile([128, 4 * T], F16, tag="krBt", name="krBt")
            vBt = pp.tile([128, NSI * NB * C], F16, tag="vBt", name="vBt")
            WoB = pp.tile([128, 4 * C], F16, tag="WoB", name="WoB")
            mskB = pp.tile([128, NU * BLK], mybir.dt.float8e4, tag="mskB", name="mskB")
            # qrT[m]: partition = c' in chunk m; free = (n 4, t 512)
            qrT = [pp.tile([128, NB * 512], F16, tag=f"qr{i}", name=f"qr{i}") for i in range(4)]
            ones = pp.tile([128, 1], F16, tag="ones", name="ones")
            ebias = pp.tile([128, 1], F32, tag="ebias", name="ebias")
            nc.vector.memset(ones, 1.0)
            nc.vector.memset(ebias, EXP_BIAS)

            def v4(ap, p=128):  # DRAM [4*p, W] -> [p, (4, W)]
                return ap.rearrange("(c p) w -> p c w", p=p)

            # deliver in consumption order: q-path, kr si-quarters and v per-si
            # chunks interleaved just-in-time; masks in halves; Wo last
            aQv = aQb.rearrange("p (c w) -> p c w", c=4)
            aQs = v4(aQ)
            WqBv = WqB.rearrange("p (c n w) -> p c n w", c=4, n=NB)
            Wqv = v4(Wq).rearrange("p c (n w) -> p c n w", n=NB)
            krv = krBt.rearrange("p (c g w) -> p c g w", c=4, g=4)
            krs = v4(krB).rearrange("p c (g w) -> p c g w", g=4)
            vBv = vB.rearrange("(g p) w -> p g w", p=128)  # [128, 16, 2048]
            vtv = vBt.rearrange("p (g w) -> p g w", g=NSI)
            mskv = mskB.rearrange("p (u w) -> p u w", u=NU)
            msks = mskd.rearrange("u p w -> p u w")

            def vdma(si):
                nc.sync.dma_start(out=vtv[:, si:si + 1, :], in_=vBv[:, si:si + 1, :])

            nc.sync.dma_start(out=aQv[:, 0:2, :], in_=aQs[:, 0:2, :])
            nc.sync.dma_start(out=WqBv[:, 0:2, 0, :], in_=Wqv[:, 0:2, 0, :])
            nc.sync.dma_start(out=aQv[:, 2:4, :], in_=aQs[:, 2:4, :])
            nc.sync.dma_start(out=WqBv[:, 2:4, 0, :], in_=Wqv[:, 2:4, 0, :])
            krv2 = krBt.rearrange("p (c g w) -> p c g w", c=4, g=8)
            krs2 = v4(krB).rearrange("p c (g w) -> p c g w", g=8)
            nc.sync.dma_start(out=krv2[:, :, 0, :], in_=krs2[:, :, 0, :])
            nc.sync.dma_start(out=WqBv[:, :, 1, :], in_=Wqv[:, :, 1, :])
            vdma(0)
            nc.sync.dma_start(out=csB.rearrange("p (c w) -> p c w", c=2), in_=v4(cosB))
            nc.sync.dma_start(out=snB.rearrange("p (c w) -> p c w", c=2), in_=v4(sinB))
            nc.sync.dma_start(out=mskv[:, 0:8, :], in_=msks[:, 0:8, :])
            vdma(1)
            nc.sync.dma_start(out=krv2[:, :, 1, :], in_=krs2[:, :, 1, :])
            nc.sync.dma_start(out=WqBv[:, :, 2, :], in_=Wqv[:, :, 2, :])
            nc.sync.dma_start(out=mskv[:, 8:20, :], in_=msks[:, 8:20, :])
            vdma(2)
            vdma(3)
            nc.sync.dma_start(out=WqBv[:, :, 3, :], in_=Wqv[:, :, 3, :])
            nc.sync.dma_start(out=krv[:, :, 1, :], in_=krs[:, :, 1, :])
            vdma(4)
            vdma(5)
            nc.sync.dma_start(out=krv[:, :, 2, :], in_=krs[:, :, 2, :])
            vdma(6)
            vdma(7)
            nc.sync.dma_start(out=mskv[:, 20:NU, :], in_=msks[:, 20:NU, :])
            vdma(8)
            vdma(9)
            nc.sync.dma_start(out=krv[:, :, 3, :], in_=krs[:, :, 3, :])
            for si in range(10, NSI):
                vdma(si)
            nc.sync.dma_start(out=WoB.rearrange("p (c w) -> p c w", c=4), in_=v4(WoD))
            aQt = [aQb[:, i * 512:(i + 1) * 512] for i in range(4)]
            WqT = [WqB[:, i * NB * C:(i + 1) * NB * C] for i in range(4)]
            cst = [csB[:, i * 512:(i + 1) * 512] for i in range(2)]
            snt = [snB[:, i * 512:(i + 1) * 512] for i in range(2)]
            krT = [krBt[:, i * T:(i + 1) * T] for i in range(4)]
            WoT = [WoB[:, i * C:(i + 1) * C] for i in range(4)]

            with tc.tile_pool(name="qpp", bufs=8, space="PSUM") as qpp:
                # ---- q proj + rope (all 4 blocks at once) ----
                for n in range(NB):
                    qp16 = [qtp.tile([128, 512], F16, tag=f"qp{m}", name=f"qp{m}")
                            for m in range(4)]
                    for m in range(4):
                        ps = qpp.tile([128, 512], F32, tag="qps", name="qps")
                        for Kc in range(4):
                            nc.tensor.matmul(
                                ps, WqT[Kc][:, (4 * n + m) * 128:(4 * n + m + 1) * 128],
                                aQt[Kc], start=(Kc == 0), stop=(Kc == 3))
                        nc.scalar.copy(out=qp16[m], in_=ps)
                    for h in range(2):
                        eng = nc.vector if h == 0 else nc.gpsimd
                        t1 = qtp.tile([128, 512], F16, tag="qt1", name="qt1")
                        t2 = qtp.tile([128, 512], F16, tag="qt2", name="qt2")
                        eng.tensor_mul(t1, qp16[h], cst[h])
                        eng.tensor_mul(t2, qp16[2 + h], snt[h])
                        nc.vector.tensor_sub(qrT[h][:, n * 512:(n + 1) * 512], t1, t2)
                        t3 = qtp.tile([128, 512], F16, tag="qt3", name="qt3")
                        t4 = qtp.tile([128, 512], F16, tag="qt4", name="qt4")
                        eng.tensor_mul(t3, qp16[h], snt[h])
                        eng.tensor_mul(t4, qp16[2 + h], cst[h])
                        nc.vector.tensor_add(qrT[2 + h][:, n * 512:(n + 1) * 512], t3, t4)

            with tc.tile_pool(name="accp", bufs=1, space="PSUM") as acc:
                yT = [acc.tile([128, 512], F32, tag=f"yT{i}", name=f"yT{i}")
                      for i in range(4)]
                Zp = acc.tile([128, 4], F32, tag="Zp", name="Zp")
                qv = [qrT[Kc].rearrange("p (n t) -> p n t", n=NB) for Kc in range(4)]
                DEPTH = 2  # PE software-pipeline depth: att(u+2) before pv(u)

                with tc.tile_pool(name="attp", bufs=3, space="PSUM") as app:
                    first_u = {sl: min(i for i, u in enumerate(UNITS) if u[1] == sl)
                               for sl in range(4)}
                    last_u = {sl: max(i for i, u in enumerate(UNITS) if u[1] == sl)
                              for sl in range(4)}

                    def emit_epilogue(sl):
                        # o = (y/Z)^T @ Wo; reuse the slot's dead yT bank
                        yb = epi.tile([128, 512], F16, tag="yb", name="yb")
                        nc.scalar.copy(out=yb, in_=yT[sl])
                        zr = epi.tile([128, 1], F32, tag="zr", name="zr")
                        nc.vector.reciprocal(zr, Zp[:, sl:sl + 1])
                        ops = acc.tile([128, 512], F32, tag=f"yT{sl}", name=f"ops{sl}")
                        for Mc in range(4):
                            nc.tensor.matmul(ops, yb[:, Mc * 128:(Mc + 1) * 128],
                                             WoT[Mc], start=(Mc == 0), stop=(Mc == 3))
                        osb = epi.tile([128, 512], F32, tag="osb", name="osb")
                        nc.vector.tensor_scalar_mul(osb, ops, zr[:, 0:1])
                        nc.sync.dma_start(out=out[sl * 128:(sl + 1) * 128, :], in_=osb)

                    def emit_zpv(ui, si, sl, p_m, cmb):
                        start = ui == first_u[sl]
                        stop = ui == last_u[sl]
                        vt = vBt[:, si * NB * C:(si + 1) * NB * C]
                        # start=True zeroes the whole PSUM bank -> only first
                        nc.tensor.matmul(Zp[:, sl:sl + 1], p_m, ones,
                                         start=(ui == 0), stop=stop)
                        for br in range(NB):
                            for Mc in range(4):
                                nc.tensor.matmul(
                                    yT[sl][:, Mc * 128:(Mc + 1) * 128],
                                    vt[:, br * 512 + Mc * 128:br * 512 + (Mc + 1) * 128],
                                    cmb[:, br * 128:(br + 1) * 128],
                                    start=(start and br == 0 and Mc == 0),
                                    stop=(stop and br == 3))
                        if stop:
                            emit_epilogue(sl)

                    pending = []
                    for ui, (si, sl) in enumerate(UNITS):
                        msk = mskB[:, ui * BLK:(ui + 1) * BLK]
                        att = app.tile([128, NB * BLK], F32, tag="att", name="att")
                        attv = att.rearrange("p (n t) -> p n t", n=NB)
                        for bp in range(2):
                            for Kc in range(4):
                                nc.tensor.matmul(
                                    attv[:, bp * 2:(bp + 1) * 2],
                                    krT[Kc][:, si * 128:(si + 1) * 128],
                                    qv[Kc][:, bp * 2:(bp + 1) * 2,
                                           sl * 128:(sl + 1) * 128],
                                    start=(bp == 0 and Kc == 0), stop=(Kc == 3))
                        e = ew.tile([128, NB * BLK], F16, tag="e", name="e")
                        nc.scalar.activation(out=e, in_=att, func=ACTF.Exp,
                                             bias=ebias[:, 0:1])
                        m1 = ew.tile([128, 256], F16, tag="m1", name="m1")
                        amax = ew.tile([128, BLK], F16, tag="amax", name="amax")
                        p_m = ew.tile([128, BLK], F16, tag="p_m", name="p_m")
                        nc.vector.tensor_max(m1, e[:, 0:256], e[:, 256:512])
                        nc.vector.tensor_max(amax, m1[:, 0:128], m1[:, 128:256])
                        nc.vector.tensor_mul(p_m, amax, msk)
                        amax_b = amax.rearrange("p (a t) -> p a t", a=1).to_broadcast(
                            [128, NB, BLK])
                        pm_b = p_m.rearrange("p (a t) -> p a t", a=1).to_broadcast(
                            [128, NB, BLK])
                        mb = ew.tile([128, NB * BLK], F16, tag="mb", name="mb")
                        cmb = ew.tile([128, NB * BLK], F16, tag="cmb", name="cmb")
                        ev = e.rearrange("p (n t) -> p n t", n=NB)
                        nc.vector.tensor_tensor(
                            out=mb.rearrange("p (n t) -> p n t", n=NB),
                            in0=ev, in1=amax_b, op=ALU.is_ge)
                        nc.vector.tensor_mul(
                            cmb.rearrange("p (n t) -> p n t", n=NB)[:, 0:2],
                            mb.rearrange("p (n t) -> p n t", n=NB)[:, 0:2],
                            pm_b[:, 0:2])
                        nc.gpsimd.tensor_mul(
                            cmb.rearrange("p (n t) -> p n t", n=NB)[:, 2:4],
                            mb.rearrange("p (n t) -> p n t", n=NB)[:, 2:4],
                            pm_b[:, 2:4])
                        pending.append((ui, si, sl, p_m, cmb))
                        if len(pending) > DEPTH:
                            emit_zpv(*pending.pop(0))
                    while pending:
                        emit_zpv(*pending.pop(0))
    nc.compile()
    _cache["b"] = nc
    return nc


def _host_prep(Wq, Wk, Wv, Wo, cos, sin):
    split_idx = np.r_[0:C:2, 1:C:2]
    Wq_p = np.ascontiguousarray(
        Wq.reshape(C, NB, C)[:, :, split_idx].reshape(C, NB * C)).astype(NPF16)
    Wk_p = (Wk[:, split_idx] * np.float32(1.0 / np.sqrt(C))).astype(NPF16)
    Wv_h = Wv.astype(NPF16)
    Wo_h = Wo.astype(NPF16)
    cosT = np.ascontiguousarray(cos[:T].T).astype(NPF16)  # [C/2, T]
    sinT = np.ascontiguousarray(sin[:T].T).astype(NPF16)
    return Wq_p, Wk_p, Wv_h, Wo_h, cosT, sinT


def _core_blocks(j):
    return [15 - j, 11 - j, 7 - j, 3 - j]


def _masks(j):
    m = np.zeros((NU, SC, BLK), np.float32)
    tbs = _core_blocks(j)
    tt = np.arange(BLK)[None, :]
    ss = np.arange(SC)[:, None]
    for ui, (si, sl) in enumerate(UNITS):
        t0 = BLK * tbs[sl]
        m[ui] = (t0 + tt) >= (SC * si + ss)
    return m.astype(ml_dtypes.float8_e4m3)


def kernel(a, x, Wq, Wk, Wv, Wo, cos, sin):
    a = np.asarray(a, np.float32)
    x = np.asarray(x, np.float32)
    Wq_p, Wk_p, Wv_h, Wo_h, cosT, sinT = _host_prep(
        np.asarray(Wq, np.float32), np.asarray(Wk, np.float32),
        np.asarray(Wv, np.float32), np.asarray(Wo, np.float32),
        np.asarray(cos, np.float32), np.asarray(sin, np.float32))

    # ---- phase A: k + v ----
    nca = build_phase_a()
    in_a = []
    for core in range(N_CORES):
        b, s4 = divmod(core, 4)
        rows = slice(512 * s4, 512 * (s4 + 1))
        in_a.append({
            "xT": np.ascontiguousarray(x[b].T[:, rows]).astype(NPF16),
            "aTv": np.ascontiguousarray(a[b].T[:, rows]).astype(NPF16),
            "Wk": Wk_p, "Wv": Wv_h,
            "cosA": np.ascontiguousarray(cosT[:, rows]),
            "sinA": np.ascontiguousarray(sinT[:, rows]),
        })
    res_a = run_bass_kernel_spmd(nca, in_a, list(range(N_CORES)))

    kr_full = [np.concatenate([res_a.results[b * 4 + s]["krA"] for s in range(4)], axis=1)
               for b in range(B)]   # [C, T] fp16
    v_full = [np.concatenate([res_a.results[b * 4 + s]["vA"] for s in range(4)], axis=0)
              for b in range(B)]    # [T, NB*C] fp16

    # ---- phase B: q + attention ----
    ncb = build_phase_b()
    in_b = []
    for core in range(N_CORES):
        b, j = divmod(core, 4)
        tcols = np.concatenate([np.arange(BLK * tb, BLK * (tb + 1))
                                for tb in _core_blocks(j)])
        in_b.append({
            "aQ": np.ascontiguousarray(a[b].T[:, tcols]).astype(NPF16),
            "Wq": Wq_p,
            "cosB": np.ascontiguousarray(cosT[:, tcols]),
            "sinB": np.ascontiguousarray(sinT[:, tcols]),
            "krB": kr_full[b],
            "vB": v_full[b],
            "Wo": Wo_h,
            "mskd": _masks(j),
        })
    res_b = run_bass_kernel_spmd(ncb, in_b, list(range(N_CORES)))

    outf = np.zeros((B, T, C), np.float32)
    for core in range(N_CORES):
        b, j = divmod(core, 4)
        o = res_b.results[core]["o"]
        for sl, tb in enumerate(_core_blocks(j)):
            outf[b, BLK * tb:BLK * (tb + 1)] = o[sl * 128:(sl + 1) * 128]
    return outf
